# revision 28
# baseline (speedup 1.0000x reference)
"""2-layer GAT (PyG GATConv x2) on 8 Trainium2 NeuronCores via Bass/Tile.

Strategy (self-contained; shapes hardcoded for the nn_GAT problem):
  - nodes split 2500/core (dst-sharded aggregation); edges (+self-loops)
    sorted by dst; per-core edge stream padded to an SPMD-uniform schedule
    of 128-edge windows grouped in 20 dst-tiles of 125 dst nodes.
  - layer 1: every core computes the full h = x@W1 table (bf16) locally
    (no collective); the src-side attention dots a_s ride the h-table rows
    as fp32; the dst-side dots a_d are expanded dst->edges on the PE via
    host-built S^T one-hot matmuls. No segment-max is needed: the logits
    are small and softmax is shift-invariant (validated vs fp64 ground
    truth).
  - aggregation: per 128-edge window, dma_gather h rows by src (the only
    indexed-DMA pass per layer - Q7 descriptor-gen is the scarce
    resource), scale by exp (DVE broadcast-mul), one-hot matmul
    (host-built S) accumulating numerator [125,512] and denominator
    [125,8] in PSUM; then divide, bias, relu.
  - layer 2: h2 = relu(out1)@W2 per dst-shard, packed with a_s2 into a
    [2500,128] bf16 table piece, AllGather'd; same window machinery with
    64 ch / 1 head; output written dst-sharded and concatenated on host.
"""

import os
import sys

sys.path.insert(0, os.path.dirname(os.path.abspath(__file__)))
try:
    import axon_shim
    axon_shim.install()
except Exception:
    pass

import numpy as np
import ml_dtypes

import concourse.bacc as bacc
import concourse.bass as bass
import concourse.mybir as mybir
import concourse.tile as tile
from concourse import library_config
from concourse.tile import add_dep_helper
from concourse.bass_utils import run_bass_kernel_spmd

F32 = mybir.dt.float32
BF16 = mybir.dt.bfloat16
I16 = mybir.dt.int16

N, E, IN, HID, HEADS, OUT = 20000, 320000, 128, 64, 8, 64
NEG = 0.2
NCORES = 8
NPC = N // NCORES          # 2500 nodes per core
TILE_D = 125               # dst nodes per tile
NT = NPC // TILE_D         # 20 tiles per core
NROWS = N + 4              # pad row N holds "neutral" values
PAD = N                    # pad row index
CH1 = HEADS * HID          # 512
HROW = 640                 # h-table row slots (bf16): 512 h | 16 (8xf32 a_s) | pad
WCH = 8                    # windows per gather chunk (1024 idx)
BIG = -1.0e4               # pad-row a_s value -> exp(lrelu(...)) == 0

DBG = int(os.environ.get("KDBG", "0"))


# ----------------------------------------------------------------- host prep
def preprocess(edge_index):
    src0 = edge_index[0].astype(np.int64)
    dst0 = edge_index[1].astype(np.int64)
    loop = np.arange(N, dtype=np.int64)
    src = np.concatenate([src0, loop])
    dst = np.concatenate([dst0, loop])
    order = np.argsort(dst, kind="stable")
    src, dst = src[order], dst[order]

    gtile = dst // TILE_D                       # global tile id, 0..159
    counts = np.bincount(gtile, minlength=NCORES * NT)
    W = np.zeros(NT, np.int64)
    for t in range(NT):
        W[t] = (counts[t::NT].max() + 127) // 128
    WOFF = np.zeros(NT + 1, np.int64)
    WOFF[1:] = np.cumsum(W)
    TW = int(WOFF[-1])
    EPAD = TW * 128

    nchunk = (TW + WCH - 1) // WCH
    chunk_w = [min(WCH, TW - c * WCH) for c in range(nchunk)]

    def idx_layout(a, cw_list):
        """pack int16 indices in per-chunk column-major-wrapped layout"""
        outb = []
        off = 0
        for cwn in cw_list:
            n_i = cwn * 128
            blk = a[off:off + n_i].astype(np.int16)
            outb.append(np.tile(blk.reshape(-1, 16).T.copy(), (8, 1)))
            off += n_i
        return np.concatenate(outb, axis=1)

    edge_off = np.zeros(NCORES * NT + 1, np.int64)
    edge_off[1:] = np.cumsum(counts)
    cores = []
    for c in range(NCORES):
        s_arr = np.full(EPAD, PAD, np.int64)
        dl_arr = np.zeros(EPAD, np.int64)
        real = np.zeros(EPAD, np.bool_)
        for t in range(NT):
            g = c * NT + t
            cnt = counts[g]
            base = WOFF[t] * 128
            sl = slice(edge_off[g], edge_off[g + 1])
            s_arr[base:base + cnt] = src[sl]
            dl_arr[base:base + cnt] = dst[sl] - (c * NPC + t * TILE_D)
            real[base:base + cnt] = True
        pos = np.arange(EPAD)
        # S: [128, TW*128] bf16, S[p, g*128 + dloc] = 1 (pads too: exp==0)
        S = np.zeros((128, TW * 128), ml_dtypes.bfloat16)
        S[pos % 128, (pos // 128) * 128 + dl_arr] = 1.0
        # S^T (f32): ST[dloc, g*128 + p] = 1 for REAL edges only
        ST = np.zeros((128, TW * 128), ml_dtypes.bfloat16)
        ST[dl_arr[real], (pos[real] // 128) * 128 + (pos[real] % 128)] = 1.0
        # own-range adtab gather rows: per tile 125 rows + 3 dummies
        adrows = np.zeros(NT * 128, np.int64)
        for t in range(NT):
            adrows[t * 128:t * 128 + TILE_D] = c * NPC + t * TILE_D + np.arange(TILE_D)
        cores.append(dict(
            src_idx=idx_layout(s_arr, chunk_w),
            adrows_idx=idx_layout(adrows, [8, 8, 4]),
            S=S, ST=ST,
        ))
    sched = dict(W=W, WOFF=WOFF, TW=TW, nchunk=nchunk, chunk_w=chunk_w)
    return sched, cores


# --------------------------------------------------------------- bass program
def build_program(sched):
    W, WOFF, TW = sched["W"], sched["WOFF"], sched["TW"]
    nchunk, chunk_w = sched["nchunk"], sched["chunk_w"]
    win_tile = np.zeros(TW, np.int64)
    for t in range(NT):
        win_tile[WOFF[t]:WOFF[t + 1]] = t
    first_win = set(int(WOFF[t]) for t in range(NT))
    last_win = set(int(WOFF[t + 1] - 1) for t in range(NT))

    nc = bacc.Bacc("TRN2", target_bir_lowering=False, debug=False,
                   num_devices=NCORES)

    # I/O
    x_in = nc.dram_tensor("x_bf16", [N, IN], BF16, kind="ExternalInput")
    W1_in = nc.dram_tensor("W1", [IN, CH1], F32, kind="ExternalInput")
    W2_in = nc.dram_tensor("W2r", [4, 128, OUT], F32, kind="ExternalInput")
    att1_in = nc.dram_tensor("att1", [16, HID], F32, kind="ExternalInput")
    att2_in = nc.dram_tensor("att2", [2, OUT], F32, kind="ExternalInput")
    b1_in = nc.dram_tensor("b1", [1, CH1], F32, kind="ExternalInput")
    b2_in = nc.dram_tensor("b2", [1, OUT], F32, kind="ExternalInput")
    ident_in = nc.dram_tensor("ident", [128, 128], BF16, kind="ExternalInput")
    srcidx_in = nc.dram_tensor("src_idx", [128, TW * 8], I16, kind="ExternalInput")
    adrows_in = nc.dram_tensor("adrows_idx", [128, NT * 8], I16, kind="ExternalInput")
    S_in = nc.dram_tensor("S", [128, TW * 128], BF16, kind="ExternalInput")
    ST_in = nc.dram_tensor("ST", [128, TW * 128], BF16, kind="ExternalInput")
    y_out = nc.dram_tensor("y", [NPC, OUT], F32, kind="ExternalOutput")

    # internal DRAM
    htab = nc.dram_tensor("htab", [NROWS, HROW], BF16)
    adtab = nc.dram_tensor("adtab", [N, 64], F32)
    t2piece = nc.dram_tensor("t2piece", [NPC, 128], BF16)
    t2full = nc.dram_tensor("t2full", [NROWS, 128], BF16, addr_space="Shared")
    if DBG:
        dbg_h = nc.dram_tensor("dbg_h", [256, HROW], BF16, kind="ExternalOutput")
        dbg_t2 = nc.dram_tensor("dbg_t2", [256, 128], BF16, kind="ExternalOutput")

    with tile.TileContext(nc, num_cores=NCORES) as tc:
        nc.gpsimd.load_library(library_config.mlp)
        with (
            tc.tile_pool(name="const", bufs=1) as constp,
            tc.tile_pool(name="work", bufs=2) as workp,
            tc.tile_pool(name="big", bufs=1) as bigp,
        ):
            # ---------------- phase 0: constants / setup ----------------
            w1f = constp.tile([128, CH1], F32, tag="w1f")
            nc.sync.dma_start(w1f[:], W1_in[:])
            w1b = constp.tile([128, CH1], BF16, tag="w1b")
            nc.vector.tensor_copy(w1b[:], w1f[:])
            w2b = constp.tile([128, 4, OUT], BF16, tag="w2b")
            w2f = workp.tile([128, 4, OUT], F32, tag="w2f")
            nc.sync.dma_start(w2f[:], W2_in.ap().rearrange("k p n -> p k n"))
            nc.vector.tensor_copy(w2b[:], w2f[:])
            identb = constp.tile([128, 128], BF16, tag="identb")
            nc.sync.dma_start(identb[:], ident_in[:])
            b1row = workp.tile([1, CH1], F32, tag="b1row")
            nc.sync.dma_start(b1row[:], b1_in[:])
            b1bc = constp.tile([128, CH1], F32, tag="b1bc")
            nc.gpsimd.partition_broadcast(b1bc[:], b1row[:])
            b2row = workp.tile([1, OUT], F32, tag="b2row")
            nc.sync.dma_start(b2row[:], b2_in[:])
            b2bc = constp.tile([128, OUT], F32, tag="b2bc")
            nc.gpsimd.partition_broadcast(b2bc[:], b2row[:])
            att2sb = constp.tile([128, OUT], F32, tag="att2sb")
            a2st = workp.tile([1, OUT], F32, tag="a2st")
            nc.sync.dma_start(a2st[:], att2_in[0:1, :])
            nc.gpsimd.partition_broadcast(att2sb[:], a2st[:])
            att2db = constp.tile([128, OUT], F32, tag="att2db")
            a2st2 = workp.tile([1, OUT], F32, tag="a2st2")
            nc.sync.dma_start(a2st2[:], att2_in[1:2, :])
            nc.gpsimd.partition_broadcast(att2db[:], a2st2[:])

            # W_att [128, 16] = per-head reductions of W1 * att1
            wattf = constp.tile([128, 16], F32, tag="wattf")
            attb = workp.tile([128, HID], F32, tag="attb")
            wtmp = workp.tile([128, HID], F32, tag="wtmp")
            for j in range(16):
                h = j % 8
                a1st = workp.tile([1, HID], F32, tag="a1st")
                nc.sync.dma_start(a1st[:], att1_in[j:j + 1, :])
                nc.gpsimd.partition_broadcast(attb[:], a1st[:])
                nc.vector.tensor_mul(wtmp[:], w1f[:, h * HID:(h + 1) * HID], attb[:])
                nc.vector.tensor_reduce(
                    wattf[:, j:j + 1], wtmp[:], op=mybir.AluOpType.add,
                    axis=mybir.AxisListType.X)
            wattb = constp.tile([128, 16], BF16, tag="wattb")
            nc.vector.tensor_copy(wattb[:], wattf[:])

            # pad rows: h=0, a_s=BIG
            zrow = workp.tile([4, HROW], BF16, tag="zrow")
            nc.vector.memset(zrow[:], 0.0)
            nc.vector.memset(zrow[:, 512:528].bitcast(F32), BIG)
            zw = nc.sync.dma_start(htab[PAD:PAD + 4, :], zrow[:])
            prow = workp.tile([4, 128], BF16, tag="prow")
            nc.vector.memset(prow[:], 0.0)
            nc.vector.memset(prow[:, 64:72].bitcast(F32), BIG)
            nc.sync.dma_start(t2full[PAD:PAD + 4, :], prow[:])

            srcidx = bigp.tile([128, TW * 8], I16, tag="srcidx")
            nc.sync.dma_start(srcidx[:], srcidx_in[:])
            adrows = bigp.tile([128, NT * 8], I16, tag="adrows")
            nc.sync.dma_start(adrows[:], adrows_in[:])
            a2all = bigp.tile([TILE_D, NT, 2], F32, tag="a2all")
            adall = bigp.tile([128, NT, 64], F32, tag="adall")

            # ------------- phase 1: full h-table (+a_s) + adtab -------------
            NXT = (N + 127) // 128
            h_writes = [zw.ins]
            ad_writes = []
            with (
                tc.tile_pool(name="pro", bufs=3) as prop,
                tc.tile_pool(name="props", bufs=2, space="PSUM") as propp,
            ):
                for i0 in range(0, NXT, 4):
                    kk = min(4, NXT - i0)
                    r0 = i0 * 128
                    nrg = min(4 * 128, N - r0)
                    xq = prop.tile([128, 4, IN], BF16, tag="xq")
                    if nrg == kk * 128:
                        nc.sync.dma_start(
                            xq[:, :kk, :],
                            x_in[r0:r0 + nrg, :].rearrange("(k p) c -> p k c", p=128))
                    else:
                        nc.sync.dma_start(xq[:nrg, 0, :], x_in[r0:r0 + nrg, :])
                    hsb = prop.tile([128, 4, 528], BF16, tag="hsb")
                    asb = prop.tile([128, 4, 16], F32, tag="asb")
                    for j in range(kk):
                        nr = min(128, N - (i0 + j) * 128)
                        ps_t = propp.tile([128, 128], BF16, tag="ps_t")
                        nc.tensor.transpose(ps_t[:, :nr], xq[:nr, j, :], identb[:nr, :nr])
                        xT = prop.tile([128, 128], BF16, tag="xT")
                        nc.scalar.copy(xT[:, :nr], ps_t[:, :nr])
                        ps_h = propp.tile([128, CH1], F32, tag="ps_h")
                        nc.tensor.matmul(ps_h[:nr, :], xT[:, :nr], w1b[:], start=True, stop=True)
                        ps_a = propp.tile([128, 16], F32, tag="ps_a")
                        nc.tensor.matmul(ps_a[:nr, :], xT[:, :nr], wattb[:], start=True, stop=True)
                        nc.scalar.copy(hsb[:nr, j, 0:CH1], ps_h[:nr, :])
                        nc.vector.tensor_copy(asb[:nr, j, :], ps_a[:nr, :])
                        nc.vector.tensor_copy(hsb[:nr, j, 512:528].bitcast(F32), asb[:nr, j, 0:8])
                    if nrg == kk * 128:
                        iw = nc.sync.dma_start(
                            htab[r0:r0 + nrg, 0:528].rearrange("(k p) c -> p k c", p=128),
                            hsb[:, :kk, :])
                    else:
                        iw = nc.sync.dma_start(htab[r0:r0 + nrg, 0:528], hsb[:nrg, 0, :])
                    h_writes.append(iw.ins)
                    if nrg == kk * 128:
                        ia = nc.sync.dma_start(
                            adtab[r0:r0 + nrg, 0:16].rearrange("(k p) c -> p k c", p=128),
                            asb[:, :kk, :])
                    else:
                        ia = nc.sync.dma_start(adtab[r0:r0 + nrg, 0:16], asb[:nrg, 0, :])
                    ad_writes.append(ia.ins)

            h_done = nc.vector.nop()
            for w_ in h_writes:
                add_dep_helper(h_done.ins, w_, reason="h-table complete")
            ad_done = nc.vector.nop()
            for w_ in ad_writes:
                add_dep_helper(ad_done.ins, w_, reason="adtab complete")

            # gather own-range a rows into SBUF [128, NT, 64]
            for (ci, cwn) in enumerate([8, 8, 4]):
                nidx = cwn * 128
                ag = nc.gpsimd.dma_gather(
                    adall[:, ci * 8:ci * 8 + cwn, :], adtab[:, :],
                    adrows[:, ci * 64:ci * 64 + nidx // 16], nidx, nidx, 64)
                add_dep_helper(ag.ins, ad_done.ins, reason="adtab RAW")

            if DBG:
                d1 = nc.sync.dma_start(dbg_h[:, :], htab[1000:1256, :])
                add_dep_helper(d1.ins, h_done.ins, reason="dbg")

            # ------------- phase 3: layer-1 aggregation + h2 -------------
            t2_writes = []
            with (
                tc.tile_pool(name="l1", bufs=4) as l1p,
                tc.tile_pool(name="l1ps", bufs=2, space="PSUM") as l1ps,
                tc.tile_pool(name="l1ps2", bufs=2, space="PSUM") as l1ps2,
                tc.tile_pool(name="l1ps3", bufs=1, space="PSUM") as l1ps3,
            ):
                for ci in range(nchunk):
                    cw = chunk_w[ci]
                    nidx = cw * 128
                    g0 = ci * WCH
                    ioff = g0 * 8
                    gh = l1p.tile([128, WCH, HROW], BF16, tag="gh")
                    gi = nc.gpsimd.dma_gather(
                        gh[:, :cw, :], htab[:, :], srcidx[:, ioff:ioff + nidx // 16],
                        nidx, nidx, HROW)
                    add_dep_helper(gi.ins, h_done.ins, reason="htab RAW")
                    ssb = l1p.tile([128, WCH, 128], BF16, tag="ssb")
                    nc.sync.dma_start(ssb[:, :cw, :], S_in[:, g0 * 128:(g0 + cw) * 128])
                    stsb = l1p.tile([128, WCH, 128], BF16, tag="stsb")
                    nc.sync.dma_start(stsb[:, :cw, :], ST_in[:, g0 * 128:(g0 + cw) * 128])
                    for wi in range(cw):
                        g = g0 + wi
                        t = int(win_tile[g])
                        if g in first_win:
                            ps_o = l1ps.tile([128, CH1], F32, tag="ps_o")
                            ps_d = l1ps2.tile([128, 8], F32, tag="ps_d")
                            adb = l1p.tile([TILE_D, 8], BF16, tag="adb")
                            nc.vector.tensor_copy(adb[:], adall[:TILE_D, t, 8:16])
                        # a_d expansion: [128 e, 8] = ST_w.T @ ad[:, 8:16]
                        ps_e = l1ps2.tile([128, 8], F32, tag="ps_e")
                        nc.tensor.matmul(ps_e[:], stsb[:TILE_D, wi, :], adb[:],
                                         start=True, stop=True)
                        ew = l1p.tile([128, 8], F32, tag="ew")
                        nc.vector.tensor_add(ew[:], ps_e[:],
                                             gh[:, wi, 512:528].bitcast(F32))
                        nc.vector.scalar_tensor_tensor(
                            ew[:], ew[:], NEG, ew[:],
                            op0=mybir.AluOpType.mult, op1=mybir.AluOpType.max)
                        expw = l1p.tile([128, 8], BF16, tag="expw")
                        nc.scalar.activation(expw[:], ew[:],
                                             mybir.ActivationFunctionType.Exp)
                        msg = l1p.tile([128, CH1], BF16, tag="msg")
                        eb = expw[:].to_broadcast((128, 8, HID))
                        nc.vector.tensor_mul(
                            msg[:].rearrange("p (h c) -> p h c", h=8),
                            gh[:, wi, 0:CH1].rearrange("p (h c) -> p h c", h=8), eb)
                        st = g in first_win
                        sp = g in last_win
                        nc.tensor.matmul(ps_o[:], ssb[:, wi, :], msg[:],
                                         start=st, stop=sp)
                        nc.tensor.matmul(ps_d[:], ssb[:, wi, :], expw[:],
                                         start=st, stop=sp)
                        if sp:
                            den = l1p.tile([TILE_D, 8], F32, tag="den")
                            nc.scalar.copy(den[:], ps_d[:TILE_D, :])
                            rec = l1p.tile([TILE_D, 8], F32, tag="rec")
                            nc.vector.reciprocal(rec[:], den[:])
                            x2 = l1p.tile([TILE_D, CH1], F32, tag="x2")
                            rb = rec[:].to_broadcast((TILE_D, 8, HID))
                            nc.vector.tensor_mul(
                                x2[:].rearrange("p (h c) -> p h c", h=8),
                                ps_o[:TILE_D, :].rearrange("p (h c) -> p h c", h=8), rb)
                            nc.vector.tensor_add(x2[:], x2[:], b1bc[:TILE_D, :])
                            x2b = l1p.tile([TILE_D, CH1], BF16, tag="x2b")
                            nc.scalar.activation(x2b[:], x2[:],
                                                 mybir.ActivationFunctionType.Relu)
                            ps_h2 = l1ps3.tile([TILE_D, OUT], F32, tag="ps_h2")
                            for k in range(4):
                                ps_x2t = l1ps3.tile([128, TILE_D], BF16, tag="ps_x2t")
                                nc.tensor.transpose(
                                    ps_x2t[:], x2b[:, k * 128:(k + 1) * 128],
                                    identb[:TILE_D, :TILE_D])
                                x2t = l1p.tile([128, TILE_D], BF16, tag="x2t")
                                nc.scalar.copy(x2t[:], ps_x2t[:])
                                nc.tensor.matmul(ps_h2[:], x2t[:], w2b[:, k, :],
                                                 start=(k == 0), stop=(k == 3))
                            h2 = l1p.tile([TILE_D, OUT], F32, tag="h2")
                            nc.vector.tensor_copy(h2[:], ps_h2[:])
                            tmp = l1p.tile([TILE_D, OUT], F32, tag="tmp")
                            nc.vector.tensor_mul(tmp[:], h2[:], att2sb[:TILE_D, :])
                            nc.vector.tensor_reduce(
                                a2all[:, t, 0:1], tmp[:], op=mybir.AluOpType.add,
                                axis=mybir.AxisListType.X)
                            nc.vector.tensor_mul(tmp[:], h2[:], att2db[:TILE_D, :])
                            nc.vector.tensor_reduce(
                                a2all[:, t, 1:2], tmp[:], op=mybir.AluOpType.add,
                                axis=mybir.AxisListType.X)
                            pc = l1p.tile([TILE_D, 128], BF16, tag="pc")
                            nc.scalar.copy(pc[:, 0:OUT], h2[:])
                            nc.vector.tensor_copy(
                                pc[:, OUT:OUT + 2].bitcast(F32), a2all[:, t, 0:1])
                            tw_ = nc.sync.dma_start(
                                t2piece[t * TILE_D:(t + 1) * TILE_D, :], pc[:])
                            t2_writes.append(tw_.ins)

            t2_done = nc.vector.nop()
            for w_ in t2_writes:
                add_dep_helper(t2_done.ins, w_, reason="t2piece complete")

            # ------------- phase 4: AllGather table2 -------------
            cc = nc.gpsimd.collective_compute(
                "AllGather", mybir.AluOpType.bypass,
                replica_groups=[list(range(NCORES))],
                ins=[t2piece[:, :]], outs=[t2full[0:N, :]],
            )
            add_dep_helper(cc.ins, t2_done.ins, reason="pieces ready")
            if DBG:
                d4 = nc.sync.dma_start(dbg_t2[:, :], t2full[1000:1256, :])
                add_dep_helper(d4.ins, cc.ins, reason="dbg after AG")

            # ------------- phase 6: layer-2 aggregation -------------
            with (
                tc.tile_pool(name="l2", bufs=4) as l2p,
                tc.tile_pool(name="l2ps", bufs=2, space="PSUM") as l2ps,
                tc.tile_pool(name="l2ps2", bufs=2, space="PSUM") as l2ps2,
            ):
                for ci in range(nchunk):
                    cw = chunk_w[ci]
                    nidx = cw * 128
                    g0 = ci * WCH
                    ioff = g0 * 8
                    g2 = l2p.tile([128, WCH, 128], BF16, tag="g2")
                    gi2 = nc.gpsimd.dma_gather(
                        g2[:, :cw, :], t2full[:, :], srcidx[:, ioff:ioff + nidx // 16],
                        nidx, nidx, 128)
                    add_dep_helper(gi2.ins, cc.ins, reason="t2full RAW")
                    ssb2 = l2p.tile([128, WCH, 128], BF16, tag="ssb2")
                    nc.sync.dma_start(ssb2[:, :cw, :], S_in[:, g0 * 128:(g0 + cw) * 128])
                    stsb2 = l2p.tile([128, WCH, 128], BF16, tag="stsb2")
                    nc.sync.dma_start(stsb2[:, :cw, :], ST_in[:, g0 * 128:(g0 + cw) * 128])
                    for wi in range(cw):
                        g = g0 + wi
                        t = int(win_tile[g])
                        if g in first_win:
                            ps_o2 = l2ps.tile([128, OUT], F32, tag="ps_o2")
                            ps_d2 = l2ps2.tile([128, 1], F32, tag="ps_d2")
                            a2b = l2p.tile([TILE_D, 1], BF16, tag="a2b")
                            nc.vector.tensor_copy(a2b[:], a2all[:, t, 1:2])
                        ps_e2 = l2ps2.tile([128, 1], F32, tag="ps_e2")
                        nc.tensor.matmul(ps_e2[:], stsb2[:TILE_D, wi, :], a2b[:],
                                         start=True, stop=True)
                        e2 = l2p.tile([128, 1], F32, tag="e2")
                        nc.vector.tensor_add(e2[:], ps_e2[:],
                                             g2[:, wi, OUT:OUT + 2].bitcast(F32))
                        nc.vector.scalar_tensor_tensor(
                            e2[:], e2[:], NEG, e2[:],
                            op0=mybir.AluOpType.mult, op1=mybir.AluOpType.max)
                        x2e = l2p.tile([128, 1], BF16, tag="x2e")
                        nc.scalar.activation(x2e[:], e2[:],
                                             mybir.ActivationFunctionType.Exp)
                        msg2 = l2p.tile([128, OUT], BF16, tag="msg2")
                        e2b = x2e[:].to_broadcast((128, 1, OUT))
                        nc.vector.tensor_mul(
                            msg2[:].rearrange("p (h c) -> p h c", h=1),
                            g2[:, wi, 0:OUT].rearrange("p (h c) -> p h c", h=1), e2b)
                        st = g in first_win
                        sp = g in last_win
                        nc.tensor.matmul(ps_o2[:], ssb2[:, wi, :], msg2[:],
                                         start=st, stop=sp)
                        nc.tensor.matmul(ps_d2[:], ssb2[:, wi, :], x2e[:],
                                         start=st, stop=sp)
                        if sp:
                            den2 = l2p.tile([TILE_D, 1], F32, tag="den2")
                            nc.scalar.copy(den2[:], ps_d2[:TILE_D, :])
                            rec2 = l2p.tile([TILE_D, 1], F32, tag="rec2")
                            nc.vector.reciprocal(rec2[:], den2[:])
                            o2 = l2p.tile([TILE_D, OUT], F32, tag="o2")
                            r2b = rec2[:].to_broadcast((TILE_D, 1, OUT))
                            nc.vector.tensor_mul(
                                o2[:].rearrange("p (h c) -> p h c", h=1),
                                ps_o2[:TILE_D, :].rearrange("p (h c) -> p h c", h=1), r2b)
                            nc.vector.tensor_add(o2[:], o2[:], b2bc[:TILE_D, :])
                            nc.sync.dma_start(
                                y_out[t * TILE_D:(t + 1) * TILE_D, :], o2[:])

    nc.compile()
    return nc


# --------------------------------------------------------------------- driver
_CACHE = {}


def kernel(x, edge_index, W1, att_src1, att_dst1, b1, W2, att_src2, att_dst2, b2):
    x = np.asarray(x); edge_index = np.asarray(edge_index)
    W1 = np.asarray(W1, np.float32); W2 = np.asarray(W2, np.float32)
    att_src1 = np.asarray(att_src1, np.float32)
    att_dst1 = np.asarray(att_dst1, np.float32)
    att_src2 = np.asarray(att_src2, np.float32)
    att_dst2 = np.asarray(att_dst2, np.float32)
    b1 = np.asarray(b1, np.float32); b2 = np.asarray(b2, np.float32)

    sched, cores = preprocess(edge_index)
    if "prog" not in _CACHE:
        _CACHE["prog"] = build_program(sched)
    nc = _CACHE["prog"]

    shared = dict(
        x_bf16=x.astype(ml_dtypes.bfloat16),
        W1=W1,
        W2r=W2.reshape(4, 128, OUT),
        att1=np.concatenate([att_src1, att_dst1], axis=0),
        att2=np.concatenate([att_src2, att_dst2], axis=0),
        b1=b1.reshape(1, CH1),
        b2=b2.reshape(1, OUT),
        ident=np.eye(128, dtype=ml_dtypes.bfloat16),
    )
    in_maps = []
    for c in range(NCORES):
        m = dict(shared)
        m["src_idx"] = cores[c]["src_idx"]
        m["adrows_idx"] = cores[c]["adrows_idx"]
        m["S"] = cores[c]["S"]
        m["ST"] = cores[c]["ST"]
        in_maps.append(m)

    trace = bool(int(os.environ.get("KTRACE", "0")))
    res = run_bass_kernel_spmd(nc, in_maps, core_ids=list(range(NCORES)),
                               trace=trace)
    kernel.last_result = res
    out = np.concatenate([res.results[c]["y"] for c in range(NCORES)], axis=0)
    return out
